# revision 25
# baseline (speedup 1.0000x reference)
"""Trainium2 Bass kernel for BoundaryLoss.

loss = mean over pixels of BCE(pred_b, tgt_b) where pred_b/tgt_b are 0/1
Sobel-boundary maps of sigmoid(logits) / targets. Since both maps are
binary, the clamped BCE reduces exactly to 100 * mean(pred_b XOR tgt_b).

v13 strategy (pure data parallel over batch, 2 samples -> 8 images/core):
  - inputs quantized to bf16 AND pre-packed on the host into QUAD image
    slabs ([128, 4*4*514] = 4 images side by side per partition, each
    image 4 row-chunks of 514 cols incl halo rows and zero pad columns).
    One DMA per quad = 128 descriptors for 4 images (descriptor rate,
    ~22ns/desc/queue, is the DMA queue bottleneck): logits on the
    sync/HWDGE queue, targets on gpsimd/SWDGE in parallel, prefetched
    one quad ahead ACROSS reps and loop iterations (preamble warms
    quad 0 / first two images once).
  - sigmoid (bf16 -> bf16) on ScalarE, ONE op per image, issued two
    images ahead, enqueued after the image's first evacuation.
  - per set (126 out rows x 512 cols, both sources): 9 matmuls.
    gx = vs@X(+2) + vsn@X(0) per source; pred gy = vd2@X(+1) + vd@X(0)
    + vd@X(+2); tgt gy = vd@Y(0) + vd@Y(+1) where Y = X(0)+X(+1) is one
    DVE pre-add per tgt image ((1,2,1) = (1,1)*(1,1)); the pred side
    skips the pre-add because DVE, not PE, is the tighter engine.
  - per-set PSUM is ONE 4-bank tile [128, 2048] laid out
    gxp|gyp|gxt|gyt; evacuation+squares are ONE ScalarE Square op
    [*, 2048] (bf16 out). EVAC_PAT can route some sets' tgt half via
    VectorE copy+square instead (walrus forbids dual-PSUM-operand DVE
    ops and any GpSimd PSUM read, so those are the only two paths);
    all-ScalarE measured fastest on HW.
  - chain per set on VectorE: m = sq_x + sq_y for both sources in one
    TT add [*,1024] (strided gather of x/y halves), s = m - 0.25
    (tensor_scalar, 4x mode), prod = s_p*s_t, xor count via is_lt with
    accum_out into a per-set counts column; host sums counts.
"""
import os
import numpy as np

import concourse.bass as bass
import concourse.tile as tile
from concourse import bacc, mybir
from concourse.bass_utils import run_bass_kernel_spmd

F32 = mybir.dt.float32
BF16 = mybir.dt.bfloat16
AF = mybir.ActivationFunctionType
OP = mybir.AluOpType

B, C, H, W = 16, 4, 512, 512
N_CORES = 8
BPC = B // N_CORES          # batch entries per core
N_IMG = BPC * C             # images per core
QUAD = 4                    # images per DMA slab
N_QUAD = N_IMG // QUAD
MAIN_BLOCKS = [(0, 0, 127, 126), (126, 125, 128, 126),
               (252, 251, 128, 126), (378, 377, 128, 126)]
LEFT_IN, LEFT_OUT, LEFT_K, LEFT_M = 503, 504, 9, 8
N_SETS = N_IMG * len(MAIN_BLOCKS) + 1   # 33 sets
N_CCOL = N_SETS // 2 + 1                # one counts column per set pair
CW = 514                                 # block chunk width incl pad cols
IMGW = 4 * CW                            # image width in the packed slab
# tgt-half evacuation engine schedule, one letter per set position
# (repeating): A=ScalarE Square (part of the whole-set op), D=VectorE
# copy-from-PSUM + square (walrus forbids reading PSUM twice in one DVE
# op, and forbids GpSimd PSUM reads entirely). Pred halves always go
# through ScalarE.
EVAC_PAT = os.environ.get("BASS_EVAC_PAT", "A")


def _evac_mode(set_idx):
    c = EVAC_PAT[set_idx % len(EVAC_PAT)]
    return {"A": "act", "D": "dve", "P": "pool"}[c]


# ---------------------------------------------------------------- bands
def _band_pair(in_rows, out_rows):
    K, M = len(in_rows), len(out_rows)
    vs = np.zeros((K, M), np.float32)
    vd = np.zeros((K, M), np.float32)
    for k, ir in enumerate(in_rows):
        for m, orow in enumerate(out_rows):
            d = ir - orow
            if d == 0:
                vs[k, m] = 2.0
            elif abs(d) == 1:
                vs[k, m] = 1.0
                vd[k, m] = float(d)
    return vs, vd


def _build_band_tensor():
    """Stack all band matrices into one [128, total_cols] array.
    offsets[(key, wname)] = (col, K, M)."""
    specs = {}
    specs['b0'] = _band_pair(range(0, 127), range(0, 126))
    specs['int'] = _band_pair(range(125, 253), range(126, 252))
    K, M = LEFT_K * N_IMG, LEFT_M * N_IMG
    vs = np.zeros((K, M), np.float32)
    vd = np.zeros((K, M), np.float32)
    svs, svd = _band_pair(range(LEFT_IN, 512), range(LEFT_OUT, 512))
    for i in range(N_IMG):
        vs[i*LEFT_K:(i+1)*LEFT_K, i*LEFT_M:(i+1)*LEFT_M] = svs
        vd[i*LEFT_K:(i+1)*LEFT_K, i*LEFT_M:(i+1)*LEFT_M] = svd
    specs['left'] = (vs, vd)

    cols = []
    offsets = {}
    col = 0
    for key, (vs, vd) in specs.items():
        for wname, wmat in (("vs", vs), ("vsn", -vs), ("vd", vd),
                            ("vd2", 2.0 * vd)):
            K, M = wmat.shape
            buf = np.zeros((128, M), np.float32)
            buf[:K, :] = wmat
            cols.append(buf)
            offsets[(key, wname)] = (col, K, M)
            col += M
    return np.concatenate(cols, axis=1), offsets


_BANDS, _BOFF = _build_band_tensor()
BANDW = _BANDS.shape[1]


# ---------------------------------------------------------------- kernel
def _emit_set(nc, wsb, counts_sb, set_idx, src_p, src_t, y_p, y_t,
              cp, ct, K, M, band_key, psum_pool, sq_pool, m_pool, bias0):
    """One block set: 8 matmuls, one-op evacuation+square, xor count.
    cp: column offset of this block in the per-image tiles (src_p, y_p,
    y_t); ct: offset in the quad tgt slab (src_t only)."""
    def wap(wname):
        col, kk, mm = _BOFF[(band_key, wname)]
        assert kk == K and mm == M
        return wsb[0:K, col:col + M]
    # PSUM layout: gxp [0:512] | gyp [512:1024] | gxt [1024:1536] |
    # gyt [1536:2048]
    ps = psum_pool.tile([128, 2048], F32, tag="ps")
    # gx = vs@X(+2) + vsn@X(0) per source (weights shared across sources)
    nc.tensor.matmul(ps[0:M, 0:512], wap("vs"),
                     src_p[0:K, cp+2:cp+514], start=True, stop=False)
    nc.tensor.matmul(ps[0:M, 1024:1536], wap("vs"),
                     src_t[0:K, ct+2:ct+514], start=True, stop=False)
    nc.tensor.matmul(ps[0:M, 0:512], wap("vsn"),
                     src_p[0:K, cp:cp+512], start=False, stop=True)
    nc.tensor.matmul(ps[0:M, 1024:1536], wap("vsn"),
                     src_t[0:K, ct:ct+512], start=False, stop=True)
    # gy pred = vd2@X(+1) + vd@X(0) + vd@X(+2) (3 matmuls, no pre-add:
    # PE has slack, DVE does not); gy tgt = vd@Y(0) + vd@Y(+1) on the
    # pre-added Y (one vd weight for all four vd matmuls)
    nc.tensor.matmul(ps[0:M, 512:1024], wap("vd2"),
                     src_p[0:K, cp+1:cp+513], start=True, stop=False)
    nc.tensor.matmul(ps[0:M, 512:1024], wap("vd"),
                     src_p[0:K, cp:cp+512], start=False, stop=False)
    nc.tensor.matmul(ps[0:M, 512:1024], wap("vd"),
                     src_p[0:K, cp+2:cp+514], start=False, stop=True)
    nc.tensor.matmul(ps[0:M, 1536:2048], wap("vd"),
                     y_t[0:K, cp:cp+512], start=True, stop=False)
    nc.tensor.matmul(ps[0:M, 1536:2048], wap("vd"),
                     y_t[0:K, cp+1:cp+513], start=False, stop=True)

    # evacuation + squares: one ScalarE op for the whole set, or the
    # tgt half via VectorE copy+square on D-scheduled sets
    sq = sq_pool.tile([128, 2048], BF16, tag="sq")
    mode = _evac_mode(set_idx)
    if mode == "act":
        nc.scalar.activation(sq[0:M, :], ps[0:M, :], AF.Square,
                             bias=bias0[0:M, 0:1])
    else:
        nc.scalar.activation(sq[0:M, 0:1024], ps[0:M, 0:1024], AF.Square,
                             bias=bias0[0:M, 0:1])
        g_t = sq_pool.tile([128, 1024], BF16, tag="gt")
        nc.vector.tensor_copy(g_t[0:M, :], ps[0:M, 1024:2048])
        nc.vector.tensor_tensor(sq[0:M, 1024:2048], g_t[0:M, :],
                                g_t[0:M, :], OP.mult)

    # chain per set: m = sq_x + sq_y (both sources in one op),
    # s = m - 0.25, prod = s_p * s_t, count += (prod < 0)
    sq4 = sq.rearrange("p (n w) -> p n w", n=4)
    m_both = m_pool.tile([128, 1024], BF16, tag="mb")
    s_both = m_pool.tile([128, 1024], BF16, tag="sb")
    prod = m_pool.tile([128, 512], BF16, tag="prod")
    ind = m_pool.tile([128, 512], BF16, tag="ind")
    m2 = m_both.rearrange("p (n w) -> p n w", n=2)
    nc.vector.tensor_tensor(m2[0:M], sq4[0:M, 0::2, :], sq4[0:M, 1::2, :],
                            OP.add)
    nc.vector.tensor_scalar(s_both[0:M, :], m_both[0:M, :], 0.25, None,
                            OP.subtract, OP.bypass)
    nc.vector.tensor_tensor(prod[0:M, :], s_both[0:M, 0:512],
                            s_both[0:M, 512:1024], OP.mult)
    nc.vector.tensor_scalar(ind[0:M, :], prod[0:M, :], 0.0, None, OP.is_lt,
                            OP.add,
                            accum_out=counts_sb[0:M, set_idx:set_idx+1])


def _build_nc(repeat: int = 1, loop_reps: int = 0):
    nc = bacc.Bacc("TRN2", target_bir_lowering=False, debug=False,
                   num_devices=N_CORES,
                   num_swdge_queues=int(os.environ.get("BASS_SWQ", "1")))
    logits = nc.declare_dram_parameter(
        "logits", [repeat * N_QUAD, 128, QUAD * IMGW], BF16, isOutput=False)
    targets = nc.declare_dram_parameter(
        "targets", [repeat * N_QUAD, 128, QUAD * IMGW], BF16,
        isOutput=False)
    logits_l = nc.declare_dram_parameter(
        "logits_l", [repeat, N_IMG * LEFT_K, CW], BF16, isOutput=False)
    targets_l = nc.declare_dram_parameter(
        "targets_l", [repeat, N_IMG * LEFT_K, CW], BF16, isOutput=False)
    bands = nc.declare_dram_parameter("bands", [128, BANDW], BF16,
                                      isOutput=False)
    counts = nc.declare_dram_parameter("counts", [128, N_SETS], F32,
                                       isOutput=True)

    with tile.TileContext(nc) as tc:
        from contextlib import ExitStack
        with ExitStack() as ctx:
            consts = ctx.enter_context(tc.tile_pool(name="consts", bufs=1))
            psum_pool = ctx.enter_context(
                tc.tile_pool(name="psum", bufs=2, space="PSUM"))
            sq_pool = ctx.enter_context(tc.tile_pool(name="sqp", bufs=4))
            m_pool = ctx.enter_context(tc.tile_pool(name="mp", bufs=6))

            wsb = consts.tile([128, BANDW], BF16)
            nc.sync.dma_start(out=wsb, in_=bands[:, :])
            bias0 = consts.tile([128, 1], F32)
            nc.vector.memset(bias0, 0.0)
            counts_sb = consts.tile([128, N_SETS], F32)
            nc.vector.memset(counts_sb, 0.0)

            # quad input slabs (2-slot rotation, host-packed zeros incl
            # pad columns); per-image sigmoid/Y tiles on a 3-slot rotation
            # NBUF must divide n_slabs (8 or 16) so the wrapped cross-rep
            # prefetch lands in the right slot on every loop iteration
            NQBUF = 2
            NBUF = 4
            ltqs, ttqs = [], []
            for i in range(NQBUF):
                ltq = consts.tile([128, QUAD * IMGW], BF16, name=f"ltq{i}")
                ttq = consts.tile([128, QUAD * IMGW], BF16, name=f"ttq{i}")
                ltqs.append(ltq); ttqs.append(ttq)
            pts, yts = [], []
            for i in range(NBUF):
                pt = consts.tile([128, IMGW], BF16, name=f"pt{i}")
                yt = consts.tile([128, IMGW], BF16, name=f"yt{i}")
                # sigmoid only writes cols 1:513 of each chunk; the pad
                # columns must read as zeros in the shifted matmuls
                pt4 = pt.rearrange("p (n w) -> p n w", n=4)
                nc.vector.memset(pt4[:, :, 0:1], 0.0)
                nc.vector.memset(pt4[:, :, 513:514], 0.0)
                # block0 reads pred rows via K=127 partitions; row 127 of
                # chunk 0 is host-zeroed in logits so sigmoid output there
                # is sigma(0)=0.5 but never read by any matmul
                pts.append(pt); yts.append(yt)
            # leftover combined tiles
            lt_l = consts.tile([128, CW], BF16, name="lt_l")
            tt16_l = consts.tile([128, CW], BF16, name="tt16_l")
            pt_l = consts.tile([128, CW], BF16, name="pt_l")
            yt_l = consts.tile([128, CW], BF16, name="yt_l")
            nc.vector.memset(pt_l[:, 0:1], 0.0)
            nc.vector.memset(pt_l[:, 513:514], 0.0)

            n_slabs = repeat * N_IMG
            n_qslabs = repeat * N_QUAD

            def issue_dma_quad(qslab):
                """One contiguous [128, QUAD*IMGW] DMA per tensor:
                logits on sync/HWDGE, targets on gpsimd/SWDGE."""
                qslab = qslab % n_qslabs
                j = qslab % NQBUF
                nc.sync.dma_start(out=ltqs[j][:, :], in_=logits[qslab])
                nc.gpsimd.dma_start(out=ttqs[j][:, :], in_=targets[qslab])

            def issue_pre(slab):
                """Sigmoid (one ScalarE op) + Y pre-adds (one DVE op per
                source): Y = X(0) + X(+1), used by the 2-matmul gy."""
                slab = slab % n_slabs
                jq = (slab // QUAD) % NQBUF
                j = slab % NBUF
                ltq, ttq = ltqs[jq], ttqs[jq]
                pt, yt = pts[j], yts[j]
                lt4 = ltq.rearrange("p (n w) -> p n w", n=4 * QUAD)
                tt4 = ttq.rearrange("p (n w) -> p n w", n=4 * QUAD)
                pt4 = pt.rearrange("p (n w) -> p n w", n=4)
                yt4 = yt.rearrange("p (n w) -> p n w", n=4)
                c0 = (slab % QUAD) * 4
                nc.scalar.activation(pt4[:, :, 1:513],
                                     lt4[:, c0:c0+4, 1:513],
                                     AF.Sigmoid, bias=bias0[:, 0:1])
                nc.vector.tensor_tensor(yt4[:, :, 0:513],
                                        tt4[:, c0:c0+4, 0:513],
                                        tt4[:, c0:c0+4, 1:514], OP.add)

            # one-time warmup: quad 0 + first two images' sigmoid/Y.
            # Steady-state refills below run one quad / two images ahead
            # and wrap across reps and loop iterations (the wrapped DMAs
            # re-fetch the same addresses, which is correct: inputs are
            # loop-invariant).
            issue_dma_quad(0)
            issue_pre(0)
            issue_pre(1)

            from contextlib import nullcontext
            loop_cm = (tc.For_i(0, loop_reps, 1) if loop_reps
                       else nullcontext())
            with loop_cm:
              for rep in range(repeat):
                set_idx = 0
                for img in range(N_IMG):
                    slab = rep * N_IMG + img
                    if img % QUAD == 0:
                        issue_dma_quad(slab // QUAD + 1)
                    issue_pre(slab + 2)
                    jq = (slab // QUAD) % NQBUF
                    j = slab % NBUF
                    ttq = ttqs[jq]
                    toff = (img % QUAD) * IMGW
                    pt, yt = pts[j], yts[j]
                    for blk, (ostart, istart, K, M) in enumerate(MAIN_BLOCKS):
                        _emit_set(nc, wsb, counts_sb, set_idx, pt, ttq,
                                  None, yt, blk*CW, toff + blk*CW, K, M,
                                  'b0' if ostart == 0 else 'int',
                                  psum_pool, sq_pool, m_pool, bias0)
                        set_idx += 1

                # leftover rows of all images, block-diagonal combined set
                # (one DMA per tensor: src [8 imgs, 9 rows, 512] -> 72 parts)
                nc.sync.dma_start(out=lt_l[0:N_IMG*LEFT_K, :],
                                  in_=logits_l[rep])
                nc.gpsimd.dma_start(out=tt16_l[0:N_IMG*LEFT_K, :],
                                    in_=targets_l[rep])
                KL, ML = LEFT_K * N_IMG, LEFT_M * N_IMG
                nc.scalar.activation(pt_l[0:KL, 1:513], lt_l[0:KL, 1:513],
                                     AF.Sigmoid, bias=bias0[0:KL, 0:1])
                nc.vector.tensor_tensor(yt_l[0:KL, 0:513],
                                        tt16_l[0:KL, 0:513],
                                        tt16_l[0:KL, 1:514], OP.add)
                _emit_set(nc, wsb, counts_sb, set_idx, pt_l, tt16_l,
                          None, yt_l, 0, 0, KL, ML, 'left',
                          psum_pool, sq_pool, m_pool, bias0)

            nc.sync.dma_start(out=counts[:, :], in_=counts_sb)
    nc.compile()
    return nc


_NC = None
LAST_RESULT = None


def pack_core(arr):
    """[BPC, C, H, W] float -> ([N_QUAD, 128, QUAD*IMGW] bf16 quad slab
    layout (4 images side by side, each 4 row-chunks incl halo rows and
    zero pad cols), [N_IMG*LEFT_K, CW] leftover)."""
    import ml_dtypes
    img = np.ascontiguousarray(arr, dtype=np.float32).reshape(
        N_IMG, H, W).astype(ml_dtypes.bfloat16)
    out = np.zeros((N_IMG, 128, IMGW), ml_dtypes.bfloat16)
    out[:, 0:127, 1:513] = img[:, 0:127, :]
    for j in range(1, 4):
        r0 = 125 + 126 * (j - 1)
        out[:, :, j*CW+1:j*CW+513] = img[:, r0:r0+128, :]
    quads = np.ascontiguousarray(
        out.reshape(N_QUAD, QUAD, 128, IMGW).transpose(0, 2, 1, 3)
        .reshape(N_QUAD, 128, QUAD * IMGW))
    left = np.zeros((N_IMG * LEFT_K, CW), ml_dtypes.bfloat16)
    left[:, 1:513] = img[:, LEFT_IN:, :].reshape(N_IMG * LEFT_K, W)
    return quads, left


def kernel(logits: np.ndarray, targets: np.ndarray) -> np.ndarray:
    global _NC, LAST_RESULT
    if _NC is None:
        _NC = _build_nc()

    import ml_dtypes
    in_maps = []
    for c in range(N_CORES):
        lp, ll = pack_core(logits[c*BPC:(c+1)*BPC])
        tp, tl = pack_core(targets[c*BPC:(c+1)*BPC])
        in_maps.append({
            "logits": lp,
            "targets": tp,
            "logits_l": ll[None],
            "targets_l": tl[None],
            "bands": _BANDS.astype(ml_dtypes.bfloat16),
        })
    res = run_bass_kernel_spmd(
        _NC, in_maps, list(range(N_CORES)),
        trace=bool(os.environ.get("BASS_TRACE_KERNEL")),
    )
    LAST_RESULT = res
    total_xor = 0.0
    for r in res.results:
        total_xor += float(np.asarray(r["counts"], dtype=np.float64).sum())
    loss = 100.0 * total_xor / float(B * C * H * W)
    return np.float32(loss)
